# revision 13
# baseline (speedup 1.0000x reference)
"""Trainium2 Bass kernel for nn_PhysicsResidual (WavePINN wave-equation residual).

Per collocation point p = (t,x,y,z):
    u = MLP_128x6_tanh(p)   (4 -> 128 -> 128 x5 -> 1, tanh, linear head)
    psi = MLP_32x2_tanh(p)  (4 -> 32 -> 32 -> 1)
    d_i = diag(Hessian u)[i],  lap = d1+d2+d3
    resid = d0 - (1+psi)^2 * lap

Algorithm (per point, exact AD):
  forward:  h_k = tanh(a_k), a_k = W_k h_{k-1} + b_k, D_k = 1 - h_k^2
  backward: v_k = dU/dh_k:  vt_6 = D_6*W_out^T, vt_{k-1} = D_{k-1}*(W_k^T vt_k)
            r_k = -2 * h_k * vt_k        (= -2 h D v)
  jets:     adot_{1,i} = W1[:,i];  hdot_{k,i} = D_k * adot_{k,i};
            adot_{k+1,i} = W_{k+1} hdot_{k,i}
  d_i = sum_k sum_j r_k[j] * adot_{k,i}[j]^2     (contract over features
        via M=1 ones-matmul, PSUM-accumulated over layers)

Sharding: pure data parallel, 16384 points -> 8 cores x 2048; weights
replicated. Layout: features on partitions, points on free axis.
"""

import sys

sys.path.insert(0, "/opt/trn_rl_repo")

from contextlib import ExitStack

import numpy as np

import concourse.bacc as bacc
import concourse.bass as bass
import concourse.tile as tile
from concourse import mybir
from concourse.bass_utils import run_bass_kernel_spmd

N_CORES = 8
NPTS = 2048  # points per core
CHUNK = 512
NCHUNK = NPTS // CHUNK
W = 128  # WavePINN width
NHID = 5  # hidden-to-hidden layers (DEPTH-1)
NLAY = 6  # tanh layers
PW = 32  # psi width

F32 = mybir.dt.float32
AF = mybir.ActivationFunctionType
ALU = mybir.AluOpType


def build_nc(stage="full"):
    nc = bacc.Bacc()

    pts = nc.declare_dram_parameter("pts", [4, NPTS], F32, isOutput=False)
    w1t = nc.declare_dram_parameter("w1t", [4, W], F32, isOutput=False)
    wfwd = nc.declare_dram_parameter("wfwd", [W, NHID * W], F32, isOutput=False)
    wbwd = nc.declare_dram_parameter("wbwd", [W, NHID * W], F32, isOutput=False)
    biases = nc.declare_dram_parameter("biases", [W, NLAY], F32, isOutput=False)
    wout = nc.declare_dram_parameter("wout", [W, 1], F32, isOutput=False)
    w1cols = nc.declare_dram_parameter("w1cols", [W, 4], F32, isOutput=False)
    jl = nc.declare_dram_parameter("jl", [W, 2], F32, isOutput=False)
    ones = nc.declare_dram_parameter("ones", [W, 1], F32, isOutput=False)
    pw1t = nc.declare_dram_parameter("pw1t", [4, PW], F32, isOutput=False)
    pw2t = nc.declare_dram_parameter("pw2t", [PW, PW], F32, isOutput=False)
    pwot = nc.declare_dram_parameter("pwot", [PW, 1], F32, isOutput=False)
    pbias = nc.declare_dram_parameter("pbias", [PW, 2], F32, isOutput=False)
    pb1 = nc.declare_dram_parameter("pb1", [1, 1], F32, isOutput=False)
    resid = nc.declare_dram_parameter("resid", [NCHUNK, CHUNK], F32, isOutput=True)

    with tile.TileContext(nc) as tc, ExitStack() as ctx:
        const = ctx.enter_context(tc.tile_pool(name="const", bufs=1))
        acts = ctx.enter_context(tc.tile_pool(name="acts", bufs=2))
        work = ctx.enter_context(tc.tile_pool(name="work", bufs=2))
        ps_a = ctx.enter_context(tc.tile_pool(name="ps_a", bufs=2, space="PSUM"))
        ps_j = ctx.enter_context(tc.tile_pool(name="ps_j", bufs=1, space="PSUM"))
        ps_d = ctx.enter_context(tc.tile_pool(name="ps_d", bufs=1, space="PSUM"))

        # --- load constants/weights into SBUF ---
        def load(name_ap, shape, tag):
            t = const.tile(shape, F32, tag=tag)
            nc.sync.dma_start(out=t[:], in_=name_ap[:])
            return t

        pts_sb = load(pts, [4, NPTS], "pts")
        w1t_sb = load(w1t, [4, W], "w1t")
        wfwd_sb = load(wfwd, [W, NHID * W], "wfwd")
        wbwd_sb = load(wbwd, [W, NHID * W], "wbwd")
        bias_sb = load(biases, [W, NLAY], "biases")
        wout_sb = load(wout, [W, 1], "wout")
        w1cols_sb = load(w1cols, [W, 4], "w1cols")
        jl_sb = load(jl, [W, 2], "jl")
        ones_sb = load(ones, [W, 1], "ones")
        pw1t_sb = load(pw1t, [4, PW], "pw1t")
        pw2t_sb = load(pw2t, [PW, PW], "pw2t")
        pwot_sb = load(pwot, [PW, 1], "pwot")
        pbias_sb = load(pbias, [PW, 2], "pbias")
        pb1_sb = load(pb1, [1, 1], "pb1")

        def wf(k):  # fwd lhsT for 0-idx layer k (1..5): W_{k+1}^T
            return wfwd_sb[:, (k - 1) * W : k * W]

        def wb(k):  # bwd lhsT for 0-idx layer k (1..5): W_{k+1}
            return wbwd_sb[:, (k - 1) * W : k * W]

        for c in range(NCHUNK):
            sl = slice(c * CHUNK, (c + 1) * CHUNK)

            y_sb = acts.tile([W, NLAY, CHUNK], F32, tag="y")
            d_sb = acts.tile([W, NLAY, CHUNK], F32, tag="d")
            r_sb = acts.tile([W, NLAY, CHUNK], F32, tag="r")

            # ---- forward ----
            for k in range(NLAY):
                a_ps = ps_a.tile([W, CHUNK], F32, tag="a")
                if k == 0:
                    nc.tensor.matmul(a_ps, w1t_sb, pts_sb[:, sl], start=True, stop=True)
                else:
                    nc.tensor.matmul(a_ps, wf(k), y_sb[:, k - 1, :], start=True, stop=True)
                nc.scalar.activation(
                    y_sb[:, k, :], a_ps, AF.Tanh, bias=bias_sb[:, k : k + 1]
                )
                sq = work.tile([W, CHUNK], F32, tag="sq")
                nc.scalar.activation(sq, y_sb[:, k, :], AF.Square)
                nc.vector.tensor_scalar(
                    d_sb[:, k, :], sq, -1.0, 1.0, ALU.mult, ALU.add
                )

            if stage == "fwd":
                res_sb = work.tile([1, CHUNK], F32, tag="res")
                nc.vector.tensor_copy(res_sb, y_sb[0:1, NLAY - 1, :])
                nc.sync.dma_start(out=resid[c : c + 1, :], in_=res_sb[:])
                continue

            # ---- backward ----
            vt = work.tile([W, CHUNK], F32, tag="vt")
            nc.vector.tensor_scalar_mul(vt, d_sb[:, NLAY - 1, :], wout_sb[:, 0:1])
            nc.vector.scalar_tensor_tensor(
                r_sb[:, NLAY - 1, :], y_sb[:, NLAY - 1, :], -2.0, vt, ALU.mult, ALU.mult
            )
            for k in range(NLAY - 1, 0, -1):
                v_ps = ps_a.tile([W, CHUNK], F32, tag="a")
                nc.tensor.matmul(v_ps, wb(k), vt, start=True, stop=True)
                vt = work.tile([W, CHUNK], F32, tag="vt")
                nc.vector.tensor_tensor(vt, d_sb[:, k - 1, :], v_ps, ALU.mult)
                nc.vector.scalar_tensor_tensor(
                    r_sb[:, k - 1, :], y_sb[:, k - 1, :], -2.0, vt, ALU.mult, ALU.mult
                )

            if stage == "bwd":
                res_sb = work.tile([1, CHUNK], F32, tag="res")
                nc.vector.tensor_copy(res_sb, r_sb[0:1, 0, :])
                nc.sync.dma_start(out=resid[c : c + 1, :], in_=res_sb[:])
                continue

            # ---- jets + curvature contraction ----
            dlap_ps = ps_d.tile([1, CHUNK], F32, tag="dlap")
            dt_ps = ps_d.tile([1, CHUNK], F32, tag="dt")
            nc.tensor.matmul(
                dlap_ps, jl_sb[:, 0:1], r_sb[:, 0, :], start=True, stop=False,
                skip_group_check=True,
            )
            nc.tensor.matmul(
                dt_ps, jl_sb[:, 1:2], r_sb[:, 0, :], start=True, stop=False,
                skip_group_check=True,
            )
            hj = work.tile([W, 4, CHUNK], F32, tag="hj")
            for i in range(4):
                nc.vector.tensor_scalar_mul(
                    hj[:, i, :], d_sb[:, 0, :], w1cols_sb[:, i : i + 1]
                )
            for k in range(1, NLAY):
                aj_ps = ps_j.tile([W, 4, CHUNK], F32, tag="aj")
                for i in range(4):
                    nc.tensor.matmul(
                        aj_ps[:, i, :], wf(k), hj[:, i, :], start=True, stop=True
                    )
                sqj = work.tile([W, 4, CHUNK], F32, tag="sqj")
                nc.scalar.activation(sqj, aj_ps, AF.Square)
                if k < NLAY - 1:
                    hj = work.tile([W, 4, CHUNK], F32, tag="hj")
                    for i in range(4):
                        nc.vector.tensor_tensor(
                            hj[:, i, :], d_sb[:, k, :], aj_ps[:, i, :], ALU.mult
                        )
                p_sum = work.tile([W, CHUNK], F32, tag="p_sum")
                nc.vector.tensor_tensor(p_sum, sqj[:, 1, :], sqj[:, 2, :], ALU.add)
                nc.vector.tensor_tensor(p_sum, p_sum, sqj[:, 3, :], ALU.add)
                ulap = work.tile([W, CHUNK], F32, tag="ulap")
                nc.vector.tensor_tensor(ulap, p_sum, r_sb[:, k, :], ALU.mult)
                ut = work.tile([W, CHUNK], F32, tag="ut")
                nc.vector.tensor_tensor(ut, sqj[:, 0, :], r_sb[:, k, :], ALU.mult)
                last = k == NLAY - 1
                nc.tensor.matmul(
                    dlap_ps, ones_sb, ulap, start=False, stop=last,
                    skip_group_check=True,
                )
                nc.tensor.matmul(
                    dt_ps, ones_sb, ut, start=False, stop=last,
                    skip_group_check=True,
                )

            if stage == "jets":
                res_sb = work.tile([1, CHUNK], F32, tag="res")
                nc.vector.tensor_copy(res_sb, dt_ps)
                nc.sync.dma_start(out=resid[c : c + 1, :], in_=res_sb[:])
                continue

            # ---- psi network ----
            pp_ps = ps_a.tile([PW, CHUNK], F32, tag="a")
            nc.tensor.matmul(pp_ps, pw1t_sb, pts_sb[:, sl], start=True, stop=True)
            hp1 = work.tile([PW, CHUNK], F32, tag="hp")
            nc.scalar.activation(hp1, pp_ps, AF.Tanh, bias=pbias_sb[:, 0:1])
            pp2_ps = ps_a.tile([PW, CHUNK], F32, tag="a")
            nc.tensor.matmul(pp2_ps, pw2t_sb, hp1, start=True, stop=True)
            hp2 = work.tile([PW, CHUNK], F32, tag="hp")
            nc.scalar.activation(hp2, pp2_ps, AF.Tanh, bias=pbias_sb[:, 1:2])
            if stage == "psi1":
                res_sb = work.tile([1, CHUNK], F32, tag="res")
                nc.vector.tensor_copy(res_sb, hp2[0:1, :])
                nc.sync.dma_start(out=resid[c : c + 1, :], in_=res_sb[:])
                continue

            psi_ps = ps_a.tile([1, CHUNK], F32, tag="a")
            nc.tensor.matmul(psi_ps, pwot_sb, hp2, start=True, stop=True)
            c2 = work.tile([1, CHUNK], F32, tag="c2")
            nc.scalar.activation(c2, psi_ps, AF.Square, bias=pb1_sb[:, 0:1])

            if stage == "psi2":
                res_sb = work.tile([1, CHUNK], F32, tag="res")
                nc.vector.tensor_copy(res_sb, c2)
                nc.sync.dma_start(out=resid[c : c + 1, :], in_=res_sb[:])
                continue

            # ---- tail: resid = dt - c2*dlap ----
            dl_sb = work.tile([1, CHUNK], F32, tag="dl")
            nc.vector.tensor_copy(dl_sb, dlap_ps)
            dt_sb = work.tile([1, CHUNK], F32, tag="dtb")
            nc.vector.tensor_copy(dt_sb, dt_ps)
            m1 = work.tile([1, CHUNK], F32, tag="m1")
            nc.vector.scalar_tensor_tensor(m1, dl_sb, -1.0, c2, ALU.mult, ALU.mult)
            res_sb = work.tile([1, CHUNK], F32, tag="res")
            nc.vector.tensor_tensor(res_sb, m1, dt_sb, ALU.add)
            nc.sync.dma_start(out=resid[c : c + 1, :], in_=res_sb[:])

    return nc


_NC_CACHE = {}


def _get_nc():
    if "nc" not in _NC_CACHE:
        nc = build_nc()
        nc.finalize()
        _NC_CACHE["nc"] = nc
    return _NC_CACHE["nc"]


def make_in_maps(t, x, y, z, uW_in, ub_in, uW_hid, ub_hid, uW_out, ub_out,
                 pW_in, pb_in, pW_hid, pb_hid, pW_out, pb_out):
    f = lambda a: np.ascontiguousarray(np.asarray(a, np.float32))
    uW_in, ub_in, uW_hid, ub_hid = f(uW_in), f(ub_in), f(uW_hid), f(ub_hid)
    uW_out, pW_in, pb_in = f(uW_out), f(pW_in), f(pb_in)
    pW_hid, pb_hid, pW_out, pb_out = f(pW_hid), f(pb_hid), f(pW_out), f(pb_out)

    pts_full = np.stack([f(t), f(x), f(y), f(z)], axis=0)  # [4, 16384]

    shared = dict(
        w1t=f(uW_in.T),
        wfwd=np.concatenate([uW_hid[i].T for i in range(NHID)], axis=1),
        wbwd=np.concatenate([uW_hid[i] for i in range(NHID)], axis=1),
        biases=np.concatenate([ub_in[:, None], ub_hid.T], axis=1),
        wout=f(uW_out[0][:, None]),
        w1cols=uW_in.copy(),
        jl=np.stack([(uW_in[:, 1:4] ** 2).sum(1), uW_in[:, 0] ** 2], axis=1),
        ones=np.ones([W, 1], np.float32),
        pw1t=f(pW_in.T),
        pw2t=f(pW_hid[0].T),
        pwot=f(pW_out[0][:, None]),
        pbias=np.stack([pb_in, pb_hid[0]], axis=1),
        pb1=np.array([[pb_out[0] + 1.0]], np.float32),
    )
    shared = {k: f(v) for k, v in shared.items()}
    in_maps = []
    for cid in range(N_CORES):
        m = dict(shared)
        m["pts"] = np.ascontiguousarray(pts_full[:, cid * NPTS : (cid + 1) * NPTS])
        in_maps.append(m)
    return in_maps


def kernel(**inputs):
    in_maps = make_in_maps(**inputs)
    nc = _get_nc()
    res = run_bass_kernel_spmd(nc, in_maps, list(range(N_CORES))).results
    out = np.concatenate(
        [np.asarray(res[cid]["resid"]).reshape(-1) for cid in range(N_CORES)]
    )
    return out.astype(np.float32)


if __name__ == "__main__":
    # quick smoke: build only
    nc = build_nc()
    print("built ok:", nc)
